# revision 30
# baseline (speedup 1.0000x reference)
"""Causal multi-head self-attention with RoPE on 8 TRN2 NeuronCores.

Sharding: batch (2) x head-groups (4 groups of 4 heads) -> 8 cores.
Each core computes q/k/v projections for its 4 heads from its batch slice,
runs causal attention, and a partial o_proj against the matching Wo column
block; the host sums the 4 partials per batch (the o_proj all-reduce).

v2 structure (vs the 260us baseline):
  * ST score pairs land in ONE [128,1024] 2-bank PSUM tile (head A bank 0,
    head B bank 1) so softmax-exp is a single wide ACTIVATE per sk-tile --
    the (N+352)cyc ACT overhead is paid once, not twice.
  * Causal masking inside the diagonal 128-block is done ON THE PE: a tiny
    N=128 matmul accumulates -1000 * strict-upper-triangle into the scores
    bank (lhsT = -1000*[p>c] const, rhs = I), so exp() flushes the masked
    region to 0. No DVE mask multiplies and no extra hop in the
    ST->exp->PV chain.
  * Softmax denominator reciprocal via the single-instruction DVE op
    reciprocal_approx_fast (~51 ULP) -- the baseline's bit-exact
    nc.vector.reciprocal was 3.3us per call (53us total).
  * RoPE runs in fp16 (2x DVE mode) with both head-pairs merged per DVE op
    via [128,2,512] APs; rotate-half block swaps ride the gpsimd DMA queue.
  * q/k PSUM->SBUF evacuation casts run on the Scalar engine (it has slack;
    DVE is a co-bottleneck), V/o_proj evacuation stays on DVE.
  * y is stored fp16 (half the HBM writeback; host sums partials in fp32).
  * Cross-chunk software pipelining: projections + RoPE for chunk c+1 and
    o_proj for chunk c-1 are emitted as filler inside attention(c)'s
    ACT-bound inner loop, so the PE never idles long enough for the HAM
    clock gate to drop it to 1.2 GHz.
"""
import numpy as np

import concourse.bass as bass
import concourse.mybir as mybir
import concourse.tile as tile
from concourse import bacc
from concourse.bass_utils import run_bass_kernel_spmd

F32 = mybir.dt.float32
F16 = mybir.dt.float16
AF = mybir.ActivationFunctionType
ALU = mybir.AluOpType

DT_MM = F16

BATCH, SEQ, DM = 2, 2048, 1024
NHEAD, DK = 16, 64
NCORES = 8
GROUPS = 4           # head groups (cores per batch)
HPC = 4              # heads per core
DH = HPC * DK        # 256 head dims per core
NK = DM // 128       # 8 contraction tiles over d_model
NJ = SEQ // 512      # 4 sq chunks
ROPE_THETA = 10000.0
LOOKAHEAD = 4        # exp lead over PV (et tiles in flight)
USE_APPROX_RCP = True

TRACE = False
LAST_RESULTS = None

_NC = None


def _build():
    nc = bacc.Bacc("TRN2", target_bir_lowering=False, debug=False)

    # Steer the ACT table-set fixpoint: every activation this kernel uses
    # (Exp, Ln, Copy) lives in natural_log_exp_and_others, but the set
    # chooser picks the first set containing each function, ping-ponging
    # table loads (2.6us each) between exp_and_others and natural_log.
    # Dropping those funcs from every other set (the cached dict is shared;
    # set order and ids are unchanged) makes the combined set the only
    # candidate -> exactly one ACT_TABLE_LOAD for the whole kernel.
    from concourse.hw_specs import get_activation_tables
    for name, fns in get_activation_tables(nc.m.arch).items():
        if name != "natural_log_exp_and_others":
            for f in (AF.Exp, AF.Ln, AF.Copy, AF.Identity):
                fns.discard(f)

    xt_d = nc.dram_tensor("xt", [DM, SEQ], DT_MM, kind="ExternalInput").ap()
    wq_d = nc.dram_tensor("wq", [DM, DH], DT_MM, kind="ExternalInput").ap()
    wk_d = nc.dram_tensor("wk", [DM, DH], DT_MM, kind="ExternalInput").ap()
    wv_d = nc.dram_tensor("wv", [DM, DH], DT_MM, kind="ExternalInput").ap()
    wo_d = nc.dram_tensor("wo", [DH, DM], DT_MM, kind="ExternalInput").ap()
    cos_d = nc.dram_tensor("cosf", [128, SEQ], DT_MM, kind="ExternalInput").ap()
    sin_d = nc.dram_tensor("sinf", [128, SEQ], DT_MM, kind="ExternalInput").ap()
    y_d = nc.dram_tensor("y", [SEQ, DM], DT_MM, kind="ExternalOutput").ap()

    with tile.TileContext(nc) as tc:
        with tc.tile_pool(name="persist", bufs=1) as pp, \
             tc.tile_pool(name="tabp", bufs=2) as tabp, \
             tc.tile_pool(name="ropep", bufs=2) as ropep, \
             tc.tile_pool(name="small", bufs=3) as sp, \
             tc.tile_pool(name="etp", bufs=LOOKAHEAD + 2) as etp, \
             tc.tile_pool(name="ysp", bufs=2) as ysp, \
             tc.tile_pool(name="ps_st", bufs=2, space="PSUM") as ps_st, \
             tc.tile_pool(name="ps_ot", bufs=2, space="PSUM") as ps_ot, \
             tc.tile_pool(name="ps_pj", bufs=1, space="PSUM") as ps_pj:

            # ---- resident tensors -------------------------------------
            qt = pp.tile([128, 2 * SEQ], DT_MM, tag="qt")
            kt = pp.tile([128, 2 * SEQ], DT_MM, tag="kt")
            v_sb = pp.tile([128, 16 * (HPC * 128)], DT_MM, tag="v")
            ht = pp.tile([128, 2 * SEQ], DT_MM, tag="ht")
            wo_sb = pp.tile([128, 2 * DM], DT_MM, tag="wo")
            xt = pp.tile([128, NK * SEQ], DT_MM, tag="xt")
            wq_sb = pp.tile([128, NK * DH], DT_MM, tag="wq")
            wk_sb = pp.tile([128, NK * DH], DT_MM, tag="wk")
            wv_sb = pp.tile([128, NK * DH], DT_MM, tag="wv")
            ones64 = pp.tile([64, 512], F32, tag="ones64")
            cs_sb = pp.tile([128, SEQ], DT_MM, tag="cs_sb")
            sn_sb = pp.tile([128, SEQ], DT_MM, tag="sn_sb")
            umask = pp.tile([128, 128], DT_MM, tag="umask")
            id128 = pp.tile([128, 128], DT_MM, tag="id128")

            # ---- init + input DMAs, spread over 5 queues so chunk-0
            # criticals land in ~6us: gpsimd carries umask + xt chunk 0
            # (xt chunks 1-3 are emitted AFTER the chunk-0 prologue so the
            # RoPE swap DMAs jump the queue), sync carries wq + cos/sin,
            # scalar carries wk (idle until the first exp), tensor carries
            # wv + wo (idle until xt chunk 0 lands anyway).
            # umask[c,p] = -1000 where p > c else 0 (strict upper triangle)
            nc.gpsimd.memset(umask[:], -1000.0)
            nc.gpsimd.affine_select(out=umask[:], in_=umask[:],
                                    compare_op=ALU.is_gt, fill=0.0,
                                    base=0, pattern=[[1, 128]],
                                    channel_multiplier=-1)
            # id128 = identity
            nc.gpsimd.memset(id128[:], 1.0)
            nc.gpsimd.affine_select(out=id128[:], in_=id128[:],
                                    compare_op=ALU.is_equal, fill=0.0,
                                    base=0, pattern=[[1, 128]],
                                    channel_multiplier=-1)
            def xt_chunk_dma(c, split=False):
                for k in range(NK):
                    eng = nc.scalar if split and k % 2 else nc.gpsimd
                    eng.dma_start(
                        out=xt[:, k * SEQ + c * 512:k * SEQ + (c + 1) * 512],
                        in_=xt_d[k * 128:(k + 1) * 128,
                                 c * 512:(c + 1) * 512])

            xt_chunk_dma(0, split=True)
            for k in range(NK):
                nc.sync.dma_start(out=wq_sb[:, k * DH:(k + 1) * DH],
                                  in_=wq_d[k * 128:(k + 1) * 128, :])
                nc.scalar.dma_start(out=wk_sb[:, k * DH:(k + 1) * DH],
                                    in_=wk_d[k * 128:(k + 1) * 128, :])
            nc.sync.dma_start(out=cs_sb[:, 0:512], in_=cos_d[:, 0:512])
            nc.sync.dma_start(out=sn_sb[:, 0:512], in_=sin_d[:, 0:512])

            # ones columns of the PV stationary operand (denominator trick)
            # on the otherwise-idle DVE queue
            vv_all = v_sb[:].rearrange("p (t d) -> p t d", d=128)
            nc.vector.memset(vv_all[:, :, 64:128], 1.0)
            nc.vector.memset(ones64[:], 1.0)

            def emit_xt_rest():
                for k in range(NK):
                    nc.gpsimd.dma_start(out=wv_sb[:, k * DH:(k + 1) * DH],
                                        in_=wv_d[k * 128:(k + 1) * 128, :])
                nc.sync.dma_start(out=cs_sb[:, 512:SEQ],
                                  in_=cos_d[:, 512:SEQ])
                nc.sync.dma_start(out=sn_sb[:, 512:SEQ],
                                  in_=sin_d[:, 512:SEQ])
                nc.sync.dma_start(
                    out=wo_sb[:].rearrange("p (k c) -> p k c", c=DM),
                    in_=wo_d[:].rearrange("(k p) c -> p k c", p=128))
                for c in range(1, NJ):
                    xt_chunk_dma(c)

            # ---- emission helpers -------------------------------------

            def qk_closures(c, pools=None):
                """Q/K projections for chunk c; 4 closures (2 per dst)."""
                out = []
                state = {}
                pools = pools or {}

                def mk_m0(dst, w_sb):
                    def f():
                        pool, tag = pools.get(id(dst), (ps_pj, "pj"))
                        ps = pool.tile([128, 1024], F32, tag=tag)
                        state[id(dst)] = ps
                        for k in range(NK):
                            nc.tensor.matmul(
                                ps[:, 0:512],
                                w_sb[:, k * DH: k * DH + 128],
                                xt[:, k * SEQ + c * 512: k * SEQ + (c + 1) * 512],
                                start=(k == 0), stop=(k == NK - 1))
                    return f

                def mk_m1(dst, w_sb):
                    def f():
                        ps = state.pop(id(dst))
                        for k in range(NK):
                            nc.tensor.matmul(
                                ps[:, 512:1024],
                                w_sb[:, k * DH + 128: k * DH + 256],
                                xt[:, k * SEQ + c * 512: k * SEQ + (c + 1) * 512],
                                start=(k == 0), stop=(k == NK - 1))
                        dview = dst[:].rearrange("p (m s) -> p m s", m=2)
                        nc.vector.tensor_copy(
                            dview[:, :, c * 512:(c + 1) * 512],
                            ps[:].rearrange("p (m s) -> p m s", m=2))
                    return f

                for dst, w_sb in ((qt, wq_sb), (kt, wk_sb)):
                    out.append(mk_m0(dst, w_sb))
                    out.append(mk_m1(dst, w_sb))
                return out

            def v_closures(c, alternate=False, cast_on_scalar=False):
                out = []

                def mk(t):
                    def f():
                        pool, tag = ((ps_st, "st") if alternate and t % 2 else
                                     (ps_pj, "pj"))
                        ps = pool.tile([128, 1024], F32, tag=tag)
                        for k in range(NK):
                            nc.tensor.matmul(
                                ps[:, 0:DH],
                                xt[:, k * SEQ + t * 128: k * SEQ + t * 128 + 128],
                                wv_sb[:, k * DH:(k + 1) * DH],
                                start=(k == 0), stop=(k == NK - 1))
                        vv = v_sb[:, t * 512:(t + 1) * 512].rearrange(
                            "p (h d) -> p h d", d=128)
                        eng = nc.scalar if cast_on_scalar else nc.vector
                        eng_copy = (nc.scalar.copy if cast_on_scalar
                                    else nc.vector.tensor_copy)
                        eng_copy(
                            vv[:, :, 0:64],
                            ps[:, 0:DH].rearrange("p (h d) -> p h d", d=64))
                    return f

                for t in range(4 * c, 4 * c + 4):
                    out.append(mk(t))
                return out

            def rope_closures(c):
                """RoPE on QT/KT chunk c in place, fp16, both m merged."""
                out = []

                cs_v = cs_sb[:, c * 512:(c + 1) * 512].unsqueeze(1) \
                    .broadcast_to([128, 2, 512])
                sn_v = sn_sb[:, c * 512:(c + 1) * 512].unsqueeze(1) \
                    .broadcast_to([128, 2, 512])

                def mk(src):
                    def f():
                        sview = src[:].rearrange("p (m s) -> p m s", m=2)
                        seg = sview[:, :, c * 512:(c + 1) * 512]
                        t1 = ropep.tile([128, 1024], DT_MM, tag="t1")
                        t1v = t1[:].rearrange("p (m s) -> p m s", m=2)
                        nc.vector.tensor_mul(t1v[:], seg, cs_v)
                        sw = ropep.tile([128, 1024], DT_MM, tag="sw")
                        swv = sw[:].rearrange("p (m s) -> p m s", m=2)
                        for blk in range(4):
                            sb_ = blk ^ 1
                            nc.gpsimd.dma_start(
                                out=swv[blk * 32:(blk + 1) * 32, :, :],
                                in_=seg[sb_ * 32:(sb_ + 1) * 32, :, :])
                        sw2 = ropep.tile([128, 1024], DT_MM, tag="sw2")
                        sw2v = sw2[:].rearrange("p (m s) -> p m s", m=2)
                        nc.vector.tensor_mul(sw2v[:], swv[:], sn_v)
                        nc.vector.tensor_add(seg, t1v[:], sw2v[:])
                    return f

                out.append(lambda: None)  # spacing placeholder
                for src in (qt, kt):
                    out.append(mk(src))
                return out

            def oproj_closures(j, tail=False):
                out = []

                def mk(t4):
                    def f():
                        # in the tail the ST pool is free: alternate pools
                        # so matmul groups overlap the PSUM-evacuation casts
                        pool, tag = ((ps_st, "st") if tail and t4 % 2 else
                                     (ps_pj, "pj"))
                        ps = pool.tile([128, 1024], F32, tag=tag)
                        for kk in range(2):
                            for n in range(2):
                                nc.tensor.matmul(
                                    ps[:, n * 512:(n + 1) * 512],
                                    ht[:, kk * SEQ + j * 512 + t4 * 128:
                                       kk * SEQ + j * 512 + (t4 + 1) * 128],
                                    wo_sb[:, kk * DM + n * 512:
                                          kk * DM + (n + 1) * 512],
                                    start=(kk == 0), stop=(kk == 1))
                        ys = ysp.tile([128, 1024], DT_MM, tag="ys")
                        nc.vector.tensor_copy(ys[:], ps[:])
                        nc.sync.dma_start(
                            out=y_d[j * 512 + t4 * 128: j * 512 + (t4 + 1) * 128, :],
                            in_=ys[:])
                    return f

                for t4 in range(4):
                    out.append(mk(t4))
                return out

            # ---- chunk 0 prologue (ST pool is still free: alternate
            # PSUM pools so projection groups overlap their casts) --------
            for f in qk_closures(0, pools={id(kt): (ps_st, "st")}):
                f()
            for f in rope_closures(0):
                f()
            emit_xt_rest()

            # ---- main pipeline ----------------------------------------
            for c in range(NJ):
                # stage work for chunk c+1 / o_proj c-1 as attn(c) filler
                filler = []
                if c == 0:
                    filler += v_closures(0, cast_on_scalar=True)
                if c < NJ - 1:
                    filler += qk_closures(c + 1)
                    filler += rope_closures(c + 1)
                    filler += v_closures(c + 1)
                # o_proj gates nothing downstream: defer it into the
                # ACT-heavy late chunks (attn(3) absorbs two of them)
                if c == 1:
                    filler += oproj_closures(0)
                elif c == 3:
                    filler += oproj_closures(1) + oproj_closures(2)

                j = c
                nlive = 4 * (j + 1)
                stride = max(1, (2 * nlive) // max(1, len(filler)))
                for hp in range(2):
                    otA = ps_ot.tile([128, 512], F32, tag="ot")
                    otB = ps_ot.tile([128, 512], F32, tag="ot")
                    jb = hp * SEQ + j * 512
                    ets = {}

                    def emit_st_exp(i, jb=jb, hp=hp, j=j, ets=ets):
                        r = i - 4 * j          # >= 0 on diagonal tiles
                        c0 = 128 * r if r >= 0 else 0
                        ib = hp * SEQ + i * 128
                        st = ps_st.tile([128, 1024], F32, tag="st")
                        nc.tensor.matmul(st[:, c0:512],
                                         kt[0:64, ib:ib + 128],
                                         qt[0:64, jb + c0:jb + 512],
                                         start=True, stop=(r < 0))
                        nc.tensor.matmul(st[:, 512 + c0:1024],
                                         kt[64:128, ib:ib + 128],
                                         qt[64:128, jb + c0:jb + 512],
                                         start=True, stop=(r < 0))
                        if r >= 0:  # -1000 * strict-upper-tri into diag block
                            nc.tensor.matmul(st[:, c0:c0 + 128],
                                             umask[:], id128[:],
                                             start=False, stop=True)
                            nc.tensor.matmul(st[:, 512 + c0:512 + c0 + 128],
                                             umask[:], id128[:],
                                             start=False, stop=True)
                        et = etp.tile([128, 1024], DT_MM, tag="et")
                        nc.scalar.activation(
                            et[:].rearrange("p (g s) -> p g s", g=2)[:, :, c0:512],
                            st[:].rearrange("p (g s) -> p g s", g=2)[:, :, c0:512],
                            AF.Exp, scale=0.125)
                        ets[i] = (et, c0)

                    def emit_pv(i, hp=hp, ets=ets, otA=otA, otB=otB,
                                nlive=nlive):
                        et, c0 = ets.pop(i)
                        vb = i * (HPC * 128) + 2 * hp * 128
                        nc.tensor.matmul(otA[:, c0:512],
                                         v_sb[:, vb:vb + 128],
                                         et[:, c0:512],
                                         start=(i == 0), stop=(i == nlive - 1))
                        nc.tensor.matmul(otB[:, c0:512],
                                         v_sb[:, vb + 128:vb + 256],
                                         et[:, 512 + c0:1024],
                                         start=(i == 0), stop=(i == nlive - 1))

                    for i in range(min(LOOKAHEAD, nlive)):
                        emit_st_exp(i)
                    for i in range(nlive):
                        hold = 2 if c == NJ - 1 else 0
                        tail_slot = (c == NJ - 1 and hp == 1
                                     and i >= nlive - 2)
                        if filler and (tail_slot or (
                                len(filler) > hold
                                and (hp * nlive + i) % stride == 0)):
                            filler.pop(0)()
                        emit_pv(i)
                        if i + LOOKAHEAD < nlive:
                            emit_st_exp(i + LOOKAHEAD)

                    # normalize: rows 0..63 / denominator (rows 64..127)
                    for sub, ot in ((0, otA), (1, otB)):
                        # 1/D: split between engines by phase -- ln+exp on
                        # the Scalar engine (1.4us, one resident table set)
                        # while ACT has slack; bit-exact DVE reciprocal
                        # (4us) in the late ACT-paced chunks
                        rcp = sp.tile([64, 512], F32, tag="rcp")
                        if c == 3 or (c == 2 and hp == 1 and sub == 1):
                            nc.vector.reciprocal(rcp[:], ot[64:128, :])
                        else:
                            lnt = sp.tile([64, 512], F32, tag="lnt")
                            nc.scalar.activation(lnt[:], ot[64:128, :],
                                                 AF.Ln)
                            nc.scalar.activation(rcp[:], lnt[:], AF.Exp,
                                                 scale=-1.0)
                        if sub == 0:
                            nc.vector.tensor_mul(ht[0:64, jb:jb + 512],
                                                 ot[0:64, :], rcp[:])
                        else:
                            stg = sp.tile([64, 512], DT_MM, tag="stg")
                            nc.vector.tensor_mul(stg[:], ot[0:64, :], rcp[:])
                            nc.sync.dma_start(out=ht[64:128, jb:jb + 512],
                                               in_=stg[:])
                # leftover filler runs PE-dense between chunks
                for f in filler:
                    f()
            for f in oproj_closures(NJ - 1, tail=True):
                f()

    nc.compile()
    return nc


def _prep_inputs(x, Wq, Wk, Wv, Wo, token_positions):
    x = np.asarray(x, dtype=np.float32)
    Wq = np.asarray(Wq, dtype=np.float32)
    Wk = np.asarray(Wk, dtype=np.float32)
    Wv = np.asarray(Wv, dtype=np.float32)
    Wo = np.asarray(Wo, dtype=np.float32)
    pos = np.asarray(token_positions).astype(np.float32)

    inv = 1.0 / (ROPE_THETA ** (np.arange(0, DK, 2, dtype=np.float32) / DK))
    freqs = pos[:, None] * inv[None, :]              # [SEQ, 32]
    cos_t, sin_t = np.cos(freqs).T, np.sin(freqs).T  # [32, SEQ]
    cosf = np.ascontiguousarray(np.tile(cos_t, (4, 1)), dtype=np.float16)
    sinf = np.tile(sin_t, (4, 1)).astype(np.float32)
    sinf[0:32] *= -1.0   # evens block gets -sin; odds +sin
    sinf[64:96] *= -1.0
    sinf = np.ascontiguousarray(sinf.astype(np.float16))

    perm = np.concatenate([np.arange(0, 64, 2), np.arange(1, 64, 2)])
    in_maps = []
    for c in range(NCORES):
        b, g = divmod(c, GROUPS)
        rows = slice(g * DH, (g + 1) * DH)
        wq_s = Wq[rows, :].reshape(HPC, DK, DM)[:, perm, :].reshape(DH, DM)
        wk_s = Wk[rows, :].reshape(HPC, DK, DM)[:, perm, :].reshape(DH, DM)
        in_maps.append({
            "xt": np.ascontiguousarray(x[b].T, dtype=np.float16),
            "wq": np.ascontiguousarray(wq_s.T, dtype=np.float16),
            "wk": np.ascontiguousarray(wk_s.T, dtype=np.float16),
            "wv": np.ascontiguousarray(Wv[rows, :].T, dtype=np.float16),
            "wo": np.ascontiguousarray(Wo[:, rows].T, dtype=np.float16),
            "cosf": cosf,
            "sinf": sinf,
        })
    return in_maps


def kernel(x, Wq, Wk, Wv, Wo, token_positions):
    global _NC, LAST_RESULTS
    if _NC is None:
        _NC = _build()
    in_maps = _prep_inputs(x, Wq, Wk, Wv, Wo, token_positions)
    res = run_bass_kernel_spmd(_NC, in_maps, list(range(NCORES)), trace=TRACE)
    LAST_RESULTS = res
    y = np.empty((BATCH, SEQ, DM), dtype=np.float32)
    for b in range(BATCH):
        acc = res.results[4 * b]["y"].astype(np.float32)
        for g in range(1, GROUPS):
            acc = acc + res.results[4 * b + g]["y"].astype(np.float32)
        y[b] = acc
    return y


# revision 31
# speedup vs baseline: 1.0276x; 1.0276x over previous
"""Causal multi-head self-attention with RoPE on 8 TRN2 NeuronCores.

Sharding: batch (2) x head-groups (4 groups of 4 heads) -> 8 cores.
Each core computes q/k/v projections for its 4 heads from its batch slice,
runs causal attention, and a partial o_proj against the matching Wo column
block; the host sums the 4 partials per batch (the o_proj all-reduce).

v2 structure (vs the 260us baseline):
  * ST score pairs land in ONE [128,1024] 2-bank PSUM tile (head A bank 0,
    head B bank 1) so softmax-exp is a single wide ACTIVATE per sk-tile --
    the (N+352)cyc ACT overhead is paid once, not twice.
  * Causal masking inside the diagonal 128-block is done ON THE PE: a tiny
    N=128 matmul accumulates -1000 * strict-upper-triangle into the scores
    bank (lhsT = -1000*[p>c] const, rhs = I), so exp() flushes the masked
    region to 0. No DVE mask multiplies and no extra hop in the
    ST->exp->PV chain.
  * Softmax denominator reciprocal via the single-instruction DVE op
    reciprocal_approx_fast (~51 ULP) -- the baseline's bit-exact
    nc.vector.reciprocal was 3.3us per call (53us total).
  * RoPE runs in fp16 (2x DVE mode) with both head-pairs merged per DVE op
    via [128,2,512] APs; rotate-half block swaps ride the gpsimd DMA queue.
  * q/k PSUM->SBUF evacuation casts run on the Scalar engine (it has slack;
    DVE is a co-bottleneck), V/o_proj evacuation stays on DVE.
  * y is stored fp16 (half the HBM writeback; host sums partials in fp32).
  * Cross-chunk software pipelining: projections + RoPE for chunk c+1 and
    o_proj for chunk c-1 are emitted as filler inside attention(c)'s
    ACT-bound inner loop, so the PE never idles long enough for the HAM
    clock gate to drop it to 1.2 GHz.
"""
import numpy as np

import concourse.bass as bass
import concourse.mybir as mybir
import concourse.tile as tile
from concourse import bacc
from concourse.bass_utils import run_bass_kernel_spmd

F32 = mybir.dt.float32
F16 = mybir.dt.float16
AF = mybir.ActivationFunctionType
ALU = mybir.AluOpType

DT_MM = F16

BATCH, SEQ, DM = 2, 2048, 1024
NHEAD, DK = 16, 64
NCORES = 8
GROUPS = 4           # head groups (cores per batch)
HPC = 4              # heads per core
DH = HPC * DK        # 256 head dims per core
NK = DM // 128       # 8 contraction tiles over d_model
NJ = SEQ // 512      # 4 sq chunks
ROPE_THETA = 10000.0
LOOKAHEAD = 4        # exp lead over PV (et tiles in flight)
USE_APPROX_RCP = True

TRACE = False
LAST_RESULTS = None

_NC = None


def _build():
    nc = bacc.Bacc("TRN2", target_bir_lowering=False, debug=False)

    # Steer the ACT table-set fixpoint: every activation this kernel uses
    # (Exp, Ln, Copy) lives in natural_log_exp_and_others, but the set
    # chooser picks the first set containing each function, ping-ponging
    # table loads (2.6us each) between exp_and_others and natural_log.
    # Dropping those funcs from every other set (the cached dict is shared;
    # set order and ids are unchanged) makes the combined set the only
    # candidate -> exactly one ACT_TABLE_LOAD for the whole kernel.
    from concourse.hw_specs import get_activation_tables
    for name, fns in get_activation_tables(nc.m.arch).items():
        if name != "natural_log_exp_and_others":
            for f in (AF.Exp, AF.Ln, AF.Copy, AF.Identity):
                fns.discard(f)

    xt_d = nc.dram_tensor("xt", [DM, SEQ], DT_MM, kind="ExternalInput").ap()
    wq_d = nc.dram_tensor("wq", [DM, DH], DT_MM, kind="ExternalInput").ap()
    wk_d = nc.dram_tensor("wk", [DM, DH], DT_MM, kind="ExternalInput").ap()
    wv_d = nc.dram_tensor("wv", [DM, DH], DT_MM, kind="ExternalInput").ap()
    wo_d = nc.dram_tensor("wo", [DH, DM], DT_MM, kind="ExternalInput").ap()
    cos_d = nc.dram_tensor("cosf", [128, SEQ], DT_MM, kind="ExternalInput").ap()
    sin_d = nc.dram_tensor("sinf", [128, SEQ], DT_MM, kind="ExternalInput").ap()
    y_d = nc.dram_tensor("y", [SEQ, DM], DT_MM, kind="ExternalOutput").ap()

    with tile.TileContext(nc) as tc:
        with tc.tile_pool(name="persist", bufs=1) as pp, \
             tc.tile_pool(name="tabp", bufs=2) as tabp, \
             tc.tile_pool(name="ropep", bufs=2) as ropep, \
             tc.tile_pool(name="small", bufs=3) as sp, \
             tc.tile_pool(name="etp", bufs=LOOKAHEAD + 2) as etp, \
             tc.tile_pool(name="ysp", bufs=2) as ysp, \
             tc.tile_pool(name="ps_st", bufs=2, space="PSUM") as ps_st, \
             tc.tile_pool(name="ps_ot", bufs=2, space="PSUM") as ps_ot, \
             tc.tile_pool(name="ps_pj", bufs=1, space="PSUM") as ps_pj:

            # ---- resident tensors -------------------------------------
            qt = pp.tile([128, 2 * SEQ], DT_MM, tag="qt")
            kt = pp.tile([128, 2 * SEQ], DT_MM, tag="kt")
            v_sb = pp.tile([128, 16 * (HPC * 128)], DT_MM, tag="v")
            ht = pp.tile([128, 2 * SEQ], DT_MM, tag="ht")
            wo_sb = pp.tile([128, 2 * DM], DT_MM, tag="wo")
            xt = pp.tile([128, NK * SEQ], DT_MM, tag="xt")
            wq_sb = pp.tile([128, NK * DH], DT_MM, tag="wq")
            wk_sb = pp.tile([128, NK * DH], DT_MM, tag="wk")
            wv_sb = pp.tile([128, NK * DH], DT_MM, tag="wv")
            ones64 = pp.tile([64, 512], F32, tag="ones64")
            cs_sb = pp.tile([128, SEQ], DT_MM, tag="cs_sb")
            sn_sb = pp.tile([128, SEQ], DT_MM, tag="sn_sb")
            umask = pp.tile([128, 128], DT_MM, tag="umask")
            id128 = pp.tile([128, 128], DT_MM, tag="id128")

            # ---- init + input DMAs, spread over 5 queues so chunk-0
            # criticals land in ~6us: gpsimd carries umask + xt chunk 0
            # (xt chunks 1-3 are emitted AFTER the chunk-0 prologue so the
            # RoPE swap DMAs jump the queue), sync carries wq + cos/sin,
            # scalar carries wk (idle until the first exp), tensor carries
            # wv + wo (idle until xt chunk 0 lands anyway).
            # umask[c,p] = -1000 where p > c else 0 (strict upper triangle)
            nc.gpsimd.memset(umask[:], -1000.0)
            nc.gpsimd.affine_select(out=umask[:], in_=umask[:],
                                    compare_op=ALU.is_gt, fill=0.0,
                                    base=0, pattern=[[1, 128]],
                                    channel_multiplier=-1)
            # id128 = identity
            nc.gpsimd.memset(id128[:], 1.0)
            nc.gpsimd.affine_select(out=id128[:], in_=id128[:],
                                    compare_op=ALU.is_equal, fill=0.0,
                                    base=0, pattern=[[1, 128]],
                                    channel_multiplier=-1)
            def xt_chunk_dma(c, split=False):
                for k in range(NK):
                    eng = nc.scalar if split and k % 2 else nc.gpsimd
                    eng.dma_start(
                        out=xt[:, k * SEQ + c * 512:k * SEQ + (c + 1) * 512],
                        in_=xt_d[k * 128:(k + 1) * 128,
                                 c * 512:(c + 1) * 512])

            xt_chunk_dma(0, split=True)
            for k in range(NK):
                nc.sync.dma_start(out=wq_sb[:, k * DH:(k + 1) * DH],
                                  in_=wq_d[k * 128:(k + 1) * 128, :])
                nc.scalar.dma_start(out=wk_sb[:, k * DH:(k + 1) * DH],
                                    in_=wk_d[k * 128:(k + 1) * 128, :])
            nc.sync.dma_start(out=cs_sb[:, 0:512], in_=cos_d[:, 0:512])
            nc.sync.dma_start(out=sn_sb[:, 0:512], in_=sin_d[:, 0:512])

            # ones columns of the PV stationary operand (denominator trick)
            # on the otherwise-idle DVE queue
            vv_all = v_sb[:].rearrange("p (t d) -> p t d", d=128)
            nc.vector.memset(vv_all[:, :, 64:128], 1.0)
            nc.vector.memset(ones64[:], 1.0)

            def emit_xt_rest():
                for k in range(NK):
                    nc.gpsimd.dma_start(out=wv_sb[:, k * DH:(k + 1) * DH],
                                        in_=wv_d[k * 128:(k + 1) * 128, :])
                nc.sync.dma_start(out=cs_sb[:, 512:SEQ],
                                  in_=cos_d[:, 512:SEQ])
                nc.sync.dma_start(out=sn_sb[:, 512:SEQ],
                                  in_=sin_d[:, 512:SEQ])
                nc.sync.dma_start(
                    out=wo_sb[:].rearrange("p (k c) -> p k c", c=DM),
                    in_=wo_d[:].rearrange("(k p) c -> p k c", p=128))
                for c in range(1, NJ):
                    xt_chunk_dma(c)

            # ---- emission helpers -------------------------------------

            def qk_closures(c, pools=None):
                """Q/K projections for chunk c; 4 closures (2 per dst)."""
                out = []
                state = {}
                pools = pools or {}

                def mk_m0(dst, w_sb):
                    def f():
                        pool, tag = pools.get(id(dst), (ps_pj, "pj"))
                        ps = pool.tile([128, 1024], F32, tag=tag)
                        state[id(dst)] = ps
                        for k in range(NK):
                            nc.tensor.matmul(
                                ps[:, 0:512],
                                w_sb[:, k * DH: k * DH + 128],
                                xt[:, k * SEQ + c * 512: k * SEQ + (c + 1) * 512],
                                start=(k == 0), stop=(k == NK - 1))
                    return f

                def mk_m1(dst, w_sb):
                    def f():
                        ps = state.pop(id(dst))
                        for k in range(NK):
                            nc.tensor.matmul(
                                ps[:, 512:1024],
                                w_sb[:, k * DH + 128: k * DH + 256],
                                xt[:, k * SEQ + c * 512: k * SEQ + (c + 1) * 512],
                                start=(k == 0), stop=(k == NK - 1))
                        dview = dst[:].rearrange("p (m s) -> p m s", m=2)
                        nc.vector.tensor_copy(
                            dview[:, :, c * 512:(c + 1) * 512],
                            ps[:].rearrange("p (m s) -> p m s", m=2))
                    return f

                for dst, w_sb in ((qt, wq_sb), (kt, wk_sb)):
                    out.append(mk_m0(dst, w_sb))
                    out.append(mk_m1(dst, w_sb))
                return out

            def v_closures(c, alternate=False, cast_on_scalar=False):
                out = []

                def mk(t):
                    def f():
                        pool, tag = ((ps_st, "st") if alternate and t % 2 else
                                     (ps_pj, "pj"))
                        ps = pool.tile([128, 1024], F32, tag=tag)
                        for k in range(NK):
                            nc.tensor.matmul(
                                ps[:, 0:DH],
                                xt[:, k * SEQ + t * 128: k * SEQ + t * 128 + 128],
                                wv_sb[:, k * DH:(k + 1) * DH],
                                start=(k == 0), stop=(k == NK - 1))
                        vv = v_sb[:, t * 512:(t + 1) * 512].rearrange(
                            "p (h d) -> p h d", d=128)
                        eng = nc.scalar if cast_on_scalar else nc.vector
                        eng_copy = (nc.scalar.copy if cast_on_scalar
                                    else nc.vector.tensor_copy)
                        eng_copy(
                            vv[:, :, 0:64],
                            ps[:, 0:DH].rearrange("p (h d) -> p h d", d=64))
                    return f

                for t in range(4 * c, 4 * c + 4):
                    out.append(mk(t))
                return out

            def rope_closures(c):
                """RoPE on QT/KT chunk c in place, fp16, both m merged."""
                out = []

                cs_v = cs_sb[:, c * 512:(c + 1) * 512].unsqueeze(1) \
                    .broadcast_to([128, 2, 512])
                sn_v = sn_sb[:, c * 512:(c + 1) * 512].unsqueeze(1) \
                    .broadcast_to([128, 2, 512])

                def mk(src):
                    def f():
                        sview = src[:].rearrange("p (m s) -> p m s", m=2)
                        seg = sview[:, :, c * 512:(c + 1) * 512]
                        t1 = ropep.tile([128, 1024], DT_MM, tag="t1")
                        t1v = t1[:].rearrange("p (m s) -> p m s", m=2)
                        nc.vector.tensor_mul(t1v[:], seg, cs_v)
                        sw = ropep.tile([128, 1024], DT_MM, tag="sw")
                        swv = sw[:].rearrange("p (m s) -> p m s", m=2)
                        for blk in range(4):
                            sb_ = blk ^ 1
                            nc.gpsimd.dma_start(
                                out=swv[blk * 32:(blk + 1) * 32, :, :],
                                in_=seg[sb_ * 32:(sb_ + 1) * 32, :, :])
                        sw2 = ropep.tile([128, 1024], DT_MM, tag="sw2")
                        sw2v = sw2[:].rearrange("p (m s) -> p m s", m=2)
                        nc.vector.tensor_mul(sw2v[:], swv[:], sn_v)
                        nc.vector.tensor_add(seg, t1v[:], sw2v[:])
                    return f

                out.append(lambda: None)  # spacing placeholder
                for src in (qt, kt):
                    out.append(mk(src))
                return out

            def oproj_closures(j, tail=False):
                out = []

                def mk(t4):
                    def f():
                        # in the tail the ST pool is free: alternate pools
                        # so matmul groups overlap the PSUM-evacuation casts
                        pool, tag = ((ps_st, "st") if tail and t4 % 2 else
                                     (ps_pj, "pj"))
                        ps = pool.tile([128, 1024], F32, tag=tag)
                        for kk in range(2):
                            for n in range(2):
                                nc.tensor.matmul(
                                    ps[:, n * 512:(n + 1) * 512],
                                    ht[:, kk * SEQ + j * 512 + t4 * 128:
                                       kk * SEQ + j * 512 + (t4 + 1) * 128],
                                    wo_sb[:, kk * DM + n * 512:
                                          kk * DM + (n + 1) * 512],
                                    start=(kk == 0), stop=(kk == 1))
                        ys = ysp.tile([128, 1024], DT_MM, tag="ys")
                        nc.vector.tensor_copy(ys[:], ps[:])
                        nc.sync.dma_start(
                            out=y_d[j * 512 + t4 * 128: j * 512 + (t4 + 1) * 128, :],
                            in_=ys[:])
                    return f

                for t4 in range(4):
                    out.append(mk(t4))
                return out

            # ---- chunk 0 prologue (ST pool is still free: alternate
            # PSUM pools so projection groups overlap their casts) --------
            for f in qk_closures(0, pools={id(kt): (ps_st, "st")}):
                f()
            for f in rope_closures(0):
                f()
            emit_xt_rest()

            # ---- main pipeline ----------------------------------------
            for c in range(NJ):
                # stage work for chunk c+1 / o_proj c-1 as attn(c) filler
                filler = []
                if c == 0:
                    filler += v_closures(0, cast_on_scalar=True)
                if c < NJ - 1:
                    filler += qk_closures(c + 1)
                    filler += rope_closures(c + 1)
                    filler += v_closures(c + 1)
                # o_proj gates nothing downstream: defer it into the
                # ACT-heavy late chunks (attn(3) absorbs two of them)
                if c == 1:
                    filler += oproj_closures(0)
                elif c == 3:
                    filler += oproj_closures(1) + oproj_closures(2)

                j = c
                nlive = 4 * (j + 1)
                stride = max(1, (2 * nlive) // max(1, len(filler)))
                for hp in range(2):
                    otA = ps_ot.tile([128, 512], F32, tag="ot")
                    otB = ps_ot.tile([128, 512], F32, tag="ot")
                    jb = hp * SEQ + j * 512
                    ets = {}

                    def emit_st_exp(i, jb=jb, hp=hp, j=j, ets=ets):
                        r = i - 4 * j          # >= 0 on diagonal tiles
                        c0 = 128 * r if r >= 0 else 0
                        ib = hp * SEQ + i * 128
                        st = ps_st.tile([128, 1024], F32, tag="st")
                        nc.tensor.matmul(st[:, c0:512],
                                         kt[0:64, ib:ib + 128],
                                         qt[0:64, jb + c0:jb + 512],
                                         start=True, stop=(r < 0))
                        nc.tensor.matmul(st[:, 512 + c0:1024],
                                         kt[64:128, ib:ib + 128],
                                         qt[64:128, jb + c0:jb + 512],
                                         start=True, stop=(r < 0))
                        if r >= 0:  # -1000 * strict-upper-tri into diag block
                            nc.tensor.matmul(st[:, c0:c0 + 128],
                                             umask[:], id128[:],
                                             start=False, stop=True)
                            nc.tensor.matmul(st[:, 512 + c0:512 + c0 + 128],
                                             umask[:], id128[:],
                                             start=False, stop=True)
                        et = etp.tile([128, 1024], DT_MM, tag="et")
                        nc.scalar.activation(
                            et[:].rearrange("p (g s) -> p g s", g=2)[:, :, c0:512],
                            st[:].rearrange("p (g s) -> p g s", g=2)[:, :, c0:512],
                            AF.Exp, scale=0.125)
                        ets[i] = (et, c0)

                    def emit_pv(i, hp=hp, ets=ets, otA=otA, otB=otB,
                                nlive=nlive):
                        et, c0 = ets.pop(i)
                        vb = i * (HPC * 128) + 2 * hp * 128
                        nc.tensor.matmul(otA[:, c0:512],
                                         v_sb[:, vb:vb + 128],
                                         et[:, c0:512],
                                         start=(i == 0), stop=(i == nlive - 1))
                        nc.tensor.matmul(otB[:, c0:512],
                                         v_sb[:, vb + 128:vb + 256],
                                         et[:, 512 + c0:1024],
                                         start=(i == 0), stop=(i == nlive - 1))

                    for i in range(min(LOOKAHEAD, nlive)):
                        emit_st_exp(i)
                    for i in range(nlive):
                        hold = 2 if c == NJ - 1 else 0
                        tail_slot = (c == NJ - 1 and hp == 1
                                     and i >= nlive - 2)
                        if filler and (tail_slot or (
                                len(filler) > hold
                                and (hp * nlive + i) % stride == 0)):
                            filler.pop(0)()
                        emit_pv(i)
                        if i + LOOKAHEAD < nlive:
                            emit_st_exp(i + LOOKAHEAD)

                    # normalize: rows 0..63 / denominator (rows 64..127)
                    for sub, ot in ((0, otA), (1, otB)):
                        # 1/D: split between engines by phase -- ln+exp on
                        # the Scalar engine (1.4us, one resident table set)
                        # while ACT has slack; bit-exact DVE reciprocal
                        # (4us) in the late ACT-paced chunks
                        rcp = sp.tile([64, 512], F32, tag="rcp")
                        lnt = sp.tile([64, 512], F32, tag="lnt")
                        nc.scalar.activation(lnt[:], ot[64:128, :], AF.Ln)
                        nc.scalar.activation(rcp[:], lnt[:], AF.Exp,
                                             scale=-1.0)
                        if sub == 0:
                            nc.vector.tensor_mul(ht[0:64, jb:jb + 512],
                                                 ot[0:64, :], rcp[:])
                        else:
                            stg = sp.tile([64, 512], DT_MM, tag="stg")
                            nc.vector.tensor_mul(stg[:], ot[0:64, :], rcp[:])
                            nc.sync.dma_start(out=ht[64:128, jb:jb + 512],
                                               in_=stg[:])
                # leftover filler runs PE-dense between chunks
                for f in filler:
                    f()
            for f in oproj_closures(NJ - 1, tail=True):
                f()

    nc.compile()
    return nc


def _prep_inputs(x, Wq, Wk, Wv, Wo, token_positions):
    x = np.asarray(x, dtype=np.float32)
    Wq = np.asarray(Wq, dtype=np.float32)
    Wk = np.asarray(Wk, dtype=np.float32)
    Wv = np.asarray(Wv, dtype=np.float32)
    Wo = np.asarray(Wo, dtype=np.float32)
    pos = np.asarray(token_positions).astype(np.float32)

    inv = 1.0 / (ROPE_THETA ** (np.arange(0, DK, 2, dtype=np.float32) / DK))
    freqs = pos[:, None] * inv[None, :]              # [SEQ, 32]
    cos_t, sin_t = np.cos(freqs).T, np.sin(freqs).T  # [32, SEQ]
    cosf = np.ascontiguousarray(np.tile(cos_t, (4, 1)), dtype=np.float16)
    sinf = np.tile(sin_t, (4, 1)).astype(np.float32)
    sinf[0:32] *= -1.0   # evens block gets -sin; odds +sin
    sinf[64:96] *= -1.0
    sinf = np.ascontiguousarray(sinf.astype(np.float16))

    perm = np.concatenate([np.arange(0, 64, 2), np.arange(1, 64, 2)])
    in_maps = []
    for c in range(NCORES):
        b, g = divmod(c, GROUPS)
        rows = slice(g * DH, (g + 1) * DH)
        wq_s = Wq[rows, :].reshape(HPC, DK, DM)[:, perm, :].reshape(DH, DM)
        wk_s = Wk[rows, :].reshape(HPC, DK, DM)[:, perm, :].reshape(DH, DM)
        in_maps.append({
            "xt": np.ascontiguousarray(x[b].T, dtype=np.float16),
            "wq": np.ascontiguousarray(wq_s.T, dtype=np.float16),
            "wk": np.ascontiguousarray(wk_s.T, dtype=np.float16),
            "wv": np.ascontiguousarray(Wv[rows, :].T, dtype=np.float16),
            "wo": np.ascontiguousarray(Wo[:, rows].T, dtype=np.float16),
            "cosf": cosf,
            "sinf": sinf,
        })
    return in_maps


def kernel(x, Wq, Wk, Wv, Wo, token_positions):
    global _NC, LAST_RESULTS
    if _NC is None:
        _NC = _build()
    in_maps = _prep_inputs(x, Wq, Wk, Wv, Wo, token_positions)
    res = run_bass_kernel_spmd(_NC, in_maps, list(range(NCORES)), trace=TRACE)
    LAST_RESULTS = res
    y = np.empty((BATCH, SEQ, DM), dtype=np.float32)
    for b in range(BATCH):
        acc = res.results[4 * b]["y"].astype(np.float32)
        for g in range(1, GROUPS):
            acc = acc + res.results[4 * b + g]["y"].astype(np.float32)
        y[b] = acc
    return y


# revision 32
# speedup vs baseline: 1.1032x; 1.0736x over previous
"""Causal multi-head self-attention with RoPE on 8 TRN2 NeuronCores.

Sharding: batch (2) x head-groups (4 groups of 4 heads) -> 8 cores.
Each core computes q/k/v projections for its 4 heads from its batch slice,
runs causal attention, and a partial o_proj against the matching Wo column
block; the host sums the 4 partials per batch (the o_proj all-reduce).

Structure (~170us vs the 260us v1; 78% of that is ACT+PE pacing):
  * Score pairs for the two heads of a partition-tile land in ONE
    [128,1024] 2-bank PSUM tile, so softmax-exp is a single wide ACTIVATE
    per sk-tile: the (N+352)cyc ACT overhead is paid once, not twice.
  * Causal masking inside the diagonal 128-block runs ON the PE: an N=128
    matmul accumulates -1000 * strict-upper-triangle into the scores bank
    (lhsT = -1000*[p>c], rhs = I), so exp() flushes the masked region to
    0 in fp16. No DVE mask multiplies, no extra hop in ST->exp->PV.
  * Softmax 1/denominator = exp(-ln D) on the Scalar engine (2 ops,
    ~1.4us vs 4us for DVE's iterative-divide reciprocal). The activation
    table-set dict is trimmed at build time so walrus pins the single
    natural_log_exp_and_others set: exactly one ACT_TABLE_LOAD, no
    exp<->ln table ping-pong (which costs 2.6us per switch).
  * RoPE runs in fp16 (2x DVE mode), both head-pairs merged per DVE op
    via [128,2,512] APs against stride-0-broadcast persistent cos/sin
    tables; rotate-half block swaps ride the gpsimd DMA queue.
  * y is stored fp16 (halves the writeback; host sums partials in fp32).
  * Cross-chunk software pipelining: projections + RoPE for chunk c+1 and
    o_proj for chunk c-1/c-2 are emitted as filler inside attention(c)'s
    ACT-bound inner loop so the PE stays dense and the HAM clock gate
    stays at 2.4 GHz; the final o_proj alternates between the pj and the
    freed ST PSUM pools so matmul groups overlap their evacuation casts.
  * Input DMAs are ring-assigned (gpsimd: xt + wv, sync: wq + cos/sin,
    scalar: wk + odd xt tiles of chunk 0) and per-k-tile so semaphore
    completions trickle in and the first projection matmuls start ~12us
    in; non-critical transfers (wv, wo, xt chunks 1-3) are issued behind
    the chunk-0 criticals to keep the shared DMA fabric clear.
"""
import numpy as np

import concourse.bass as bass
import concourse.mybir as mybir
import concourse.tile as tile
from concourse import bacc
from concourse.bass_utils import run_bass_kernel_spmd

F32 = mybir.dt.float32
F16 = mybir.dt.float16
AF = mybir.ActivationFunctionType
ALU = mybir.AluOpType

DT_MM = F16

BATCH, SEQ, DM = 2, 2048, 1024
NHEAD, DK = 16, 64
NCORES = 8
GROUPS = 4           # head groups (cores per batch)
HPC = 4              # heads per core
DH = HPC * DK        # 256 head dims per core
NK = DM // 128       # 8 contraction tiles over d_model
NJ = SEQ // 512      # 4 sq chunks
ROPE_THETA = 10000.0
LOOKAHEAD = 4        # exp lead over PV (et tiles in flight)

TRACE = False
LAST_RESULTS = None

_NC = None


def _build():
    nc = bacc.Bacc("TRN2", target_bir_lowering=False, debug=False)

    # Steer the ACT table-set fixpoint: every activation this kernel uses
    # (Exp, Ln, Copy) lives in natural_log_exp_and_others, but the set
    # chooser picks the first set containing each function, ping-ponging
    # table loads (2.6us each) between exp_and_others and natural_log.
    # Dropping those funcs from every other set (the cached dict is shared;
    # set order and ids are unchanged) makes the combined set the only
    # candidate -> exactly one ACT_TABLE_LOAD for the whole kernel.
    from concourse.hw_specs import get_activation_tables
    for name, fns in get_activation_tables(nc.m.arch).items():
        if name != "natural_log_exp_and_others":
            for f in (AF.Exp, AF.Ln, AF.Copy, AF.Identity):
                fns.discard(f)

    xt_d = nc.dram_tensor("xt", [DM, SEQ], DT_MM, kind="ExternalInput").ap()
    wq_d = nc.dram_tensor("wq", [DM, DH], DT_MM, kind="ExternalInput").ap()
    wk_d = nc.dram_tensor("wk", [DM, DH], DT_MM, kind="ExternalInput").ap()
    wv_d = nc.dram_tensor("wv", [DM, DH], DT_MM, kind="ExternalInput").ap()
    wo_d = nc.dram_tensor("wo", [DH, DM], DT_MM, kind="ExternalInput").ap()
    cos_d = nc.dram_tensor("cosf", [128, SEQ], DT_MM, kind="ExternalInput").ap()
    sin_d = nc.dram_tensor("sinf", [128, SEQ], DT_MM, kind="ExternalInput").ap()
    y_d = nc.dram_tensor("y", [SEQ, DM], DT_MM, kind="ExternalOutput").ap()

    with tile.TileContext(nc) as tc:
        with tc.tile_pool(name="persist", bufs=1) as pp, \
             tc.tile_pool(name="ropep", bufs=2) as ropep, \
             tc.tile_pool(name="small", bufs=3) as sp, \
             tc.tile_pool(name="etp", bufs=LOOKAHEAD + 2) as etp, \
             tc.tile_pool(name="ysp", bufs=2) as ysp, \
             tc.tile_pool(name="ps_st", bufs=2, space="PSUM") as ps_st, \
             tc.tile_pool(name="ps_ot", bufs=2, space="PSUM") as ps_ot, \
             tc.tile_pool(name="ps_pj", bufs=1, space="PSUM") as ps_pj:

            # ---- resident tensors -------------------------------------
            qt = pp.tile([128, 2 * SEQ], DT_MM, tag="qt")
            kt = pp.tile([128, 2 * SEQ], DT_MM, tag="kt")
            v_sb = pp.tile([128, 16 * (HPC * 128)], DT_MM, tag="v")
            ht = pp.tile([128, 2 * SEQ], DT_MM, tag="ht")
            wo_sb = pp.tile([128, 2 * DM], DT_MM, tag="wo")
            xt = pp.tile([128, NK * SEQ], DT_MM, tag="xt")
            wq_sb = pp.tile([128, NK * DH], DT_MM, tag="wq")
            wk_sb = pp.tile([128, NK * DH], DT_MM, tag="wk")
            wv_sb = pp.tile([128, NK * DH], DT_MM, tag="wv")
            cs_sb = pp.tile([128, SEQ], DT_MM, tag="cs_sb")
            sn_sb = pp.tile([128, SEQ], DT_MM, tag="sn_sb")
            umask = pp.tile([128, 128], DT_MM, tag="umask")
            id128 = pp.tile([128, 128], DT_MM, tag="id128")

            # ---- init + input DMAs, spread over 5 queues so chunk-0
            # criticals land in ~6us: gpsimd carries umask + xt chunk 0
            # (xt chunks 1-3 are emitted AFTER the chunk-0 prologue so the
            # RoPE swap DMAs jump the queue), sync carries wq + cos/sin,
            # scalar carries wk (idle until the first exp), tensor carries
            # wv + wo (idle until xt chunk 0 lands anyway).
            # umask[c,p] = -1000 where p > c else 0 (strict upper triangle)
            nc.gpsimd.memset(umask[:], -1000.0)
            nc.gpsimd.affine_select(out=umask[:], in_=umask[:],
                                    compare_op=ALU.is_gt, fill=0.0,
                                    base=0, pattern=[[1, 128]],
                                    channel_multiplier=-1)
            # id128 = identity
            nc.gpsimd.memset(id128[:], 1.0)
            nc.gpsimd.affine_select(out=id128[:], in_=id128[:],
                                    compare_op=ALU.is_equal, fill=0.0,
                                    base=0, pattern=[[1, 128]],
                                    channel_multiplier=-1)
            def xt_chunk_dma(c, split=False):
                for k in range(NK):
                    eng = nc.scalar if split and k % 2 else nc.gpsimd
                    eng.dma_start(
                        out=xt[:, k * SEQ + c * 512:k * SEQ + (c + 1) * 512],
                        in_=xt_d[k * 128:(k + 1) * 128,
                                 c * 512:(c + 1) * 512])

            xt_chunk_dma(0, split=True)
            for k in range(NK):
                nc.sync.dma_start(out=wq_sb[:, k * DH:(k + 1) * DH],
                                  in_=wq_d[k * 128:(k + 1) * 128, :])
                nc.scalar.dma_start(out=wk_sb[:, k * DH:(k + 1) * DH],
                                    in_=wk_d[k * 128:(k + 1) * 128, :])
            nc.sync.dma_start(out=cs_sb[:, 0:512], in_=cos_d[:, 0:512])
            nc.sync.dma_start(out=sn_sb[:, 0:512], in_=sin_d[:, 0:512])

            # ones columns of the PV stationary operand (denominator trick)
            # on the otherwise-idle DVE queue
            vv_all = v_sb[:].rearrange("p (t d) -> p t d", d=128)
            nc.vector.memset(vv_all[:, :, 64:128], 1.0)

            def emit_xt_rest():
                for k in range(NK):
                    nc.gpsimd.dma_start(out=wv_sb[:, k * DH:(k + 1) * DH],
                                        in_=wv_d[k * 128:(k + 1) * 128, :])
                nc.sync.dma_start(out=cs_sb[:, 512:SEQ],
                                  in_=cos_d[:, 512:SEQ])
                nc.sync.dma_start(out=sn_sb[:, 512:SEQ],
                                  in_=sin_d[:, 512:SEQ])
                nc.sync.dma_start(
                    out=wo_sb[:].rearrange("p (k c) -> p k c", c=DM),
                    in_=wo_d[:].rearrange("(k p) c -> p k c", p=128))
                for c in range(1, NJ):
                    xt_chunk_dma(c)

            # ---- emission helpers -------------------------------------

            def qk_closures(c, pools=None):
                """Q/K projections for chunk c; 4 closures (2 per dst)."""
                out = []
                state = {}
                pools = pools or {}

                def mk_m0(dst, w_sb):
                    def f():
                        pool, tag = pools.get(id(dst), (ps_pj, "pj"))
                        ps = pool.tile([128, 1024], F32, tag=tag)
                        state[id(dst)] = ps
                        for k in range(NK):
                            nc.tensor.matmul(
                                ps[:, 0:512],
                                w_sb[:, k * DH: k * DH + 128],
                                xt[:, k * SEQ + c * 512: k * SEQ + (c + 1) * 512],
                                start=(k == 0), stop=(k == NK - 1))
                    return f

                def mk_m1(dst, w_sb):
                    def f():
                        ps = state.pop(id(dst))
                        for k in range(NK):
                            nc.tensor.matmul(
                                ps[:, 512:1024],
                                w_sb[:, k * DH + 128: k * DH + 256],
                                xt[:, k * SEQ + c * 512: k * SEQ + (c + 1) * 512],
                                start=(k == 0), stop=(k == NK - 1))
                        dview = dst[:].rearrange("p (m s) -> p m s", m=2)
                        nc.vector.tensor_copy(
                            dview[:, :, c * 512:(c + 1) * 512],
                            ps[:].rearrange("p (m s) -> p m s", m=2))
                    return f

                for dst, w_sb in ((qt, wq_sb), (kt, wk_sb)):
                    out.append(mk_m0(dst, w_sb))
                    out.append(mk_m1(dst, w_sb))
                return out

            def v_closures(c, alternate=False, cast_on_scalar=False):
                out = []

                def mk(t):
                    def f():
                        pool, tag = ((ps_st, "st") if alternate and t % 2 else
                                     (ps_pj, "pj"))
                        ps = pool.tile([128, 1024], F32, tag=tag)
                        for k in range(NK):
                            nc.tensor.matmul(
                                ps[:, 0:DH],
                                xt[:, k * SEQ + t * 128: k * SEQ + t * 128 + 128],
                                wv_sb[:, k * DH:(k + 1) * DH],
                                start=(k == 0), stop=(k == NK - 1))
                        vv = v_sb[:, t * 512:(t + 1) * 512].rearrange(
                            "p (h d) -> p h d", d=128)
                        eng = nc.scalar if cast_on_scalar else nc.vector
                        eng_copy = (nc.scalar.copy if cast_on_scalar
                                    else nc.vector.tensor_copy)
                        eng_copy(
                            vv[:, :, 0:64],
                            ps[:, 0:DH].rearrange("p (h d) -> p h d", d=64))
                    return f

                for t in range(4 * c, 4 * c + 4):
                    out.append(mk(t))
                return out

            def rope_closures(c):
                """RoPE on QT/KT chunk c in place, fp16, both m merged."""
                out = []

                cs_v = cs_sb[:, c * 512:(c + 1) * 512].unsqueeze(1) \
                    .broadcast_to([128, 2, 512])
                sn_v = sn_sb[:, c * 512:(c + 1) * 512].unsqueeze(1) \
                    .broadcast_to([128, 2, 512])

                def mk(src):
                    def f():
                        sview = src[:].rearrange("p (m s) -> p m s", m=2)
                        seg = sview[:, :, c * 512:(c + 1) * 512]
                        t1 = ropep.tile([128, 1024], DT_MM, tag="t1")
                        t1v = t1[:].rearrange("p (m s) -> p m s", m=2)
                        nc.vector.tensor_mul(t1v[:], seg, cs_v)
                        sw = ropep.tile([128, 1024], DT_MM, tag="sw")
                        swv = sw[:].rearrange("p (m s) -> p m s", m=2)
                        for blk in range(4):
                            sb_ = blk ^ 1
                            nc.gpsimd.dma_start(
                                out=swv[blk * 32:(blk + 1) * 32, :, :],
                                in_=seg[sb_ * 32:(sb_ + 1) * 32, :, :])
                        sw2 = ropep.tile([128, 1024], DT_MM, tag="sw2")
                        sw2v = sw2[:].rearrange("p (m s) -> p m s", m=2)
                        nc.vector.tensor_mul(sw2v[:], swv[:], sn_v)
                        nc.vector.tensor_add(seg, t1v[:], sw2v[:])
                    return f

                out.append(lambda: None)  # spacing placeholder
                for src in (qt, kt):
                    out.append(mk(src))
                return out

            def oproj_closures(j, tail=False):
                out = []

                def mk(t4):
                    def f():
                        # in the tail the ST pool is free: alternate pools
                        # so matmul groups overlap the PSUM-evacuation casts
                        pool, tag = ((ps_st, "st") if tail and t4 % 2 else
                                     (ps_pj, "pj"))
                        ps = pool.tile([128, 1024], F32, tag=tag)
                        for kk in range(2):
                            for n in range(2):
                                nc.tensor.matmul(
                                    ps[:, n * 512:(n + 1) * 512],
                                    ht[:, kk * SEQ + j * 512 + t4 * 128:
                                       kk * SEQ + j * 512 + (t4 + 1) * 128],
                                    wo_sb[:, kk * DM + n * 512:
                                          kk * DM + (n + 1) * 512],
                                    start=(kk == 0), stop=(kk == 1))
                        ys = ysp.tile([128, 1024], DT_MM, tag="ys")
                        nc.vector.tensor_copy(ys[:], ps[:])
                        nc.sync.dma_start(
                            out=y_d[j * 512 + t4 * 128: j * 512 + (t4 + 1) * 128, :],
                            in_=ys[:])
                    return f

                for t4 in range(4):
                    out.append(mk(t4))
                return out

            # ---- chunk 0 prologue (ST pool is still free: alternate
            # PSUM pools so projection groups overlap their casts) --------
            for f in qk_closures(0, pools={id(kt): (ps_st, "st")}):
                f()
            for f in rope_closures(0):
                f()
            emit_xt_rest()

            # ---- main pipeline ----------------------------------------
            for c in range(NJ):
                # stage work for chunk c+1 / o_proj c-1 as attn(c) filler
                filler = []
                if c == 0:
                    filler += v_closures(0, cast_on_scalar=True)
                if c < NJ - 1:
                    filler += qk_closures(c + 1)
                    filler += rope_closures(c + 1)
                    filler += v_closures(c + 1)
                # o_proj gates nothing downstream: defer it into the
                # ACT-heavy late chunks (attn(3) absorbs two of them)
                if c == 1:
                    filler += oproj_closures(0)
                elif c == 3:
                    filler += oproj_closures(1) + oproj_closures(2)

                j = c
                nlive = 4 * (j + 1)
                stride = max(1, (2 * nlive) // max(1, len(filler)))
                for hp in range(2):
                    otA = ps_ot.tile([128, 512], F32, tag="ot")
                    otB = ps_ot.tile([128, 512], F32, tag="ot")
                    jb = hp * SEQ + j * 512
                    ets = {}

                    def emit_st_exp(i, jb=jb, hp=hp, j=j, ets=ets):
                        r = i - 4 * j          # >= 0 on diagonal tiles
                        c0 = 128 * r if r >= 0 else 0
                        ib = hp * SEQ + i * 128
                        st = ps_st.tile([128, 1024], F32, tag="st")
                        nc.tensor.matmul(st[:, c0:512],
                                         kt[0:64, ib:ib + 128],
                                         qt[0:64, jb + c0:jb + 512],
                                         start=True, stop=(r < 0))
                        nc.tensor.matmul(st[:, 512 + c0:1024],
                                         kt[64:128, ib:ib + 128],
                                         qt[64:128, jb + c0:jb + 512],
                                         start=True, stop=(r < 0))
                        if r >= 0:  # -1000 * strict-upper-tri into diag block
                            nc.tensor.matmul(st[:, c0:c0 + 128],
                                             umask[:], id128[:],
                                             start=False, stop=True)
                            nc.tensor.matmul(st[:, 512 + c0:512 + c0 + 128],
                                             umask[:], id128[:],
                                             start=False, stop=True)
                        et = etp.tile([128, 1024], DT_MM, tag="et")
                        nc.scalar.activation(
                            et[:].rearrange("p (g s) -> p g s", g=2)[:, :, c0:512],
                            st[:].rearrange("p (g s) -> p g s", g=2)[:, :, c0:512],
                            AF.Exp, scale=0.125)
                        ets[i] = (et, c0)

                    def emit_pv(i, hp=hp, ets=ets, otA=otA, otB=otB,
                                nlive=nlive):
                        et, c0 = ets.pop(i)
                        vb = i * (HPC * 128) + 2 * hp * 128
                        nc.tensor.matmul(otA[:, c0:512],
                                         v_sb[:, vb:vb + 128],
                                         et[:, c0:512],
                                         start=(i == 0), stop=(i == nlive - 1))
                        nc.tensor.matmul(otB[:, c0:512],
                                         v_sb[:, vb + 128:vb + 256],
                                         et[:, 512 + c0:1024],
                                         start=(i == 0), stop=(i == nlive - 1))

                    for i in range(min(LOOKAHEAD, nlive)):
                        emit_st_exp(i)
                    for i in range(nlive):
                        hold = 2 if c == NJ - 1 else 0
                        tail_slot = (c == NJ - 1 and hp == 1
                                     and i >= nlive - 2)
                        if filler and (tail_slot or (
                                len(filler) > hold
                                and (hp * nlive + i) % stride == 0)):
                            filler.pop(0)()
                        emit_pv(i)
                        if i + LOOKAHEAD < nlive:
                            emit_st_exp(i + LOOKAHEAD)

                    # normalize: rows 0..63 / denominator (rows 64..127)
                    for sub, ot in ((0, otA), (1, otB)):
                        # 1/D: split between engines by phase -- ln+exp on
                        # the Scalar engine (1.4us, one resident table set)
                        # while ACT has slack; bit-exact DVE reciprocal
                        # (4us) in the late ACT-paced chunks
                        rcp = sp.tile([64, 512], F32, tag="rcp")
                        lnt = sp.tile([64, 512], F32, tag="lnt")
                        nc.scalar.activation(lnt[:], ot[64:128, :], AF.Ln)
                        nc.scalar.activation(rcp[:], lnt[:], AF.Exp,
                                             scale=-1.0)
                        if sub == 0:
                            nc.vector.tensor_mul(ht[0:64, jb:jb + 512],
                                                 ot[0:64, :], rcp[:])
                        else:
                            stg = sp.tile([64, 512], DT_MM, tag="stg")
                            nc.vector.tensor_mul(stg[:], ot[0:64, :], rcp[:])
                            nc.sync.dma_start(out=ht[64:128, jb:jb + 512],
                                               in_=stg[:])
                # leftover filler runs PE-dense between chunks
                for f in filler:
                    f()
            for f in oproj_closures(NJ - 1, tail=True):
                f()

    nc.compile()
    return nc


def _prep_inputs(x, Wq, Wk, Wv, Wo, token_positions):
    x = np.asarray(x, dtype=np.float32)
    Wq = np.asarray(Wq, dtype=np.float32)
    Wk = np.asarray(Wk, dtype=np.float32)
    Wv = np.asarray(Wv, dtype=np.float32)
    Wo = np.asarray(Wo, dtype=np.float32)
    pos = np.asarray(token_positions).astype(np.float32)

    inv = 1.0 / (ROPE_THETA ** (np.arange(0, DK, 2, dtype=np.float32) / DK))
    freqs = pos[:, None] * inv[None, :]              # [SEQ, 32]
    cos_t, sin_t = np.cos(freqs).T, np.sin(freqs).T  # [32, SEQ]
    cosf = np.ascontiguousarray(np.tile(cos_t, (4, 1)), dtype=np.float16)
    sinf = np.tile(sin_t, (4, 1)).astype(np.float32)
    sinf[0:32] *= -1.0   # evens block gets -sin; odds +sin
    sinf[64:96] *= -1.0
    sinf = np.ascontiguousarray(sinf.astype(np.float16))

    perm = np.concatenate([np.arange(0, 64, 2), np.arange(1, 64, 2)])
    in_maps = []
    for c in range(NCORES):
        b, g = divmod(c, GROUPS)
        rows = slice(g * DH, (g + 1) * DH)
        wq_s = Wq[rows, :].reshape(HPC, DK, DM)[:, perm, :].reshape(DH, DM)
        wk_s = Wk[rows, :].reshape(HPC, DK, DM)[:, perm, :].reshape(DH, DM)
        in_maps.append({
            "xt": np.ascontiguousarray(x[b].T, dtype=np.float16),
            "wq": np.ascontiguousarray(wq_s.T, dtype=np.float16),
            "wk": np.ascontiguousarray(wk_s.T, dtype=np.float16),
            "wv": np.ascontiguousarray(Wv[rows, :].T, dtype=np.float16),
            "wo": np.ascontiguousarray(Wo[:, rows].T, dtype=np.float16),
            "cosf": cosf,
            "sinf": sinf,
        })
    return in_maps


def kernel(x, Wq, Wk, Wv, Wo, token_positions):
    global _NC, LAST_RESULTS
    if _NC is None:
        _NC = _build()
    in_maps = _prep_inputs(x, Wq, Wk, Wv, Wo, token_positions)
    res = run_bass_kernel_spmd(_NC, in_maps, list(range(NCORES)), trace=TRACE)
    LAST_RESULTS = res
    y = np.empty((BATCH, SEQ, DM), dtype=np.float32)
    for b in range(BATCH):
        acc = res.results[4 * b]["y"].astype(np.float32)
        for g in range(1, GROUPS):
            acc = acc + res.results[4 * b + g]["y"].astype(np.float32)
        y[b] = acc
    return y
